# revision 1
# baseline (speedup 1.0000x reference)
"""AttentionGRUCell fused kernel for 8 Trainium2 NeuronCores.

Data-parallel over the batch dim: each of the 8 cores processes a
2048-row shard of the 16384-row batch; the small weight matrices are
replicated.  Per core the cell runs in three phases, with one gate's
bf16 weight set ([128, 32, 1024] = 64 KiB/partition) resident per
phase in a persistent two-slot pool, so the next phase's weights
stream in while the current phase computes:

  phase z: load x,h,a fp32 per 128-row batch tile -> cast bf16 -> one
    DMA-XBAR transpose into k-major [128, 32, 128] -> 64 matmuls into
    PSUM -> sigmoid -> z (fp16).  xhaT and z are spilled to DRAM.
  phase r: read back xhaT, 64 matmuls, sigmoid in-place in PSUM,
    rh = r*h (bf16), transpose rh, spill rhT.
  phase s+t: lhsT = [xT | rhT | aT]; 64 matmuls for s plus the x/a
    part of t -> tanh -> s = h + z*(tanh - h) in fp32 -> store s;
    transpose s -> finish t with the s@Uo matmuls -> relu -> store t.

All matmuls use bf16 operands (stationary = transposed activation
tile, moving = 512-wide slice of the resident weights) with fp32 PSUM
accumulation.
"""

import sys

if "/opt/trn_rl_repo" not in sys.path:
    sys.path.insert(0, "/opt/trn_rl_repo")

import numpy as np

BATCH = 16384
EMB = 1024
HID = 1024
COMB = 2048
N_CORES = 8
B_L = BATCH // N_CORES          # rows per core
P = 128                         # partitions
N_BT_FULL = B_L // P            # batch tiles per core


def _build_nc(n_bt, with_bias, mm_stride=1, wload="hostbf16", layout="p3", repeat=1, phases="all", tweak=""):
    """Build + compile the per-core Bass program for n_bt batch tiles."""
    import concourse.mybir as mybir
    from contextlib import ExitStack
    from concourse import bacc
    from concourse.tile import TileContext

    dt = mybir.dt
    AF = mybir.ActivationFunctionType
    b_l = n_bt * P

    nc = bacc.Bacc("TRN2", target_bir_lowering=False, debug=False,
                   num_devices=N_CORES)

    if "cat" in tweak:
        xha_d = nc.declare_dram_parameter("xha", [b_l, EMB + HID + COMB],
                                          dt.float32, isOutput=False)
        x_d = xha_d[:, 0:EMB]
        h_d = xha_d[:, EMB:EMB + HID]
        a_d = xha_d[:, EMB + HID:]
    else:
        x_d = nc.declare_dram_parameter("x", [b_l, EMB], dt.float32,
                                        isOutput=False).ap()
        h_d = nc.declare_dram_parameter("h", [b_l, HID], dt.float32,
                                        isOutput=False).ap()
        a_d = nc.declare_dram_parameter("a", [b_l, COMB], dt.float32,
                                        isOutput=False).ap()
    wnames = ["Wz", "Uz", "Cz", "Wr", "Ur", "Cr",
              "W", "U", "C", "Vo", "Co", "Uo"]
    wshapes = {n: ([COMB, HID] if n.startswith("C") else [EMB, HID])
               for n in wnames}
    wdt = dt.bfloat16 if wload == "hostbf16" else dt.float32
    wd = {n: nc.declare_dram_parameter(n, wshapes[n], wdt, isOutput=False)
          for n in wnames}
    bias_d = {}
    if with_bias:
        for g in ("z", "r", "s", "t"):
            bias_d[g] = nc.declare_dram_parameter(
                f"bias_{g}", [P, HID], dt.float32, isOutput=False)
    s_out = nc.declare_dram_parameter("s", [b_l, HID], dt.float32, isOutput=True)
    t_out = nc.declare_dram_parameter("t", [b_l, HID], dt.float32, isOutput=True)

    # DRAM spill scratch, laid out to match the SBUF access order.
    xhaT_d = nc.dram_tensor("xhaT_spill", [n_bt, P, 32, P], dt.bfloat16)
    rhT_d = nc.dram_tensor("rhT_spill", [n_bt, P, 8, P], dt.bfloat16)
    sT_d = nc.dram_tensor("sT_spill", [n_bt, P, 8, P], dt.bfloat16)
    z_d = nc.dram_tensor("z_spill", [n_bt, P, HID], dt.float16)

    with TileContext(nc) as tc, ExitStack() as top:
        # Persistent pools: weights rotate through 2 slots so phase k+1's
        # weight DMAs+casts overlap phase k's matmuls.
        wp = top.enter_context(tc.tile_pool(name="w", bufs=2))
        stg = top.enter_context(tc.tile_pool(name="stg", bufs=2))

        def load_weights_bf16(gate_blocks, weng=None):
            wt = wp.tile([P, 32, HID], dt.bfloat16, tag="w")
            kb0 = 0
            i = 0
            for wdram, nkb in gate_blocks:
                if wload in ("gpsimd", "hostbf16"):
                    eng = nc.gpsimd if wload == "gpsimd" else (weng or nc.sync)
                    for kb in range(0, nkb, 2):
                        eng.dma_start(
                            wt[:, kb0 + kb:kb0 + kb + 2, :],
                            wdram[kb * P:(kb + 2) * P, :].rearrange(
                                "(kb p) n -> p kb n", p=P))
                    kb0 += nkb
                    continue
                for kb in range(nkb):
                    s_ = stg.tile([P, HID], dt.float32, tag="wstg")
                    nc.sync.dma_start(s_[:], wdram[kb * P:(kb + 1) * P, :])
                    if i % 2 == 0:
                        nc.vector.tensor_copy(wt[:, kb0 + kb, :], s_[:])
                    else:
                        nc.scalar.activation(wt[:, kb0 + kb, :], s_[:], AF.Copy)
                    i += 1
                kb0 += nkb
            return wt

        def load_bias(g):
            if not with_bias:
                return None
            b = wp.tile([P, HID], dt.float32, tag="bias")
            nc.sync.dma_start(b[:], bias_d[g][:])
            return b

        def mm_kloop(psums, lhs_of_kb, w, kbs, open_=True, close=True):
            kbs = list(kbs)[::mm_stride]
            first, last = kbs[0], kbs[-1]
            for kb in kbs:
                st = open_ and kb == first
                sp = close and kb == last
                lhsT = lhs_of_kb(kb)
                nc.tensor.matmul(psums[0][:], lhsT, w[:, kb, 0:512],
                                 start=st, stop=sp)
                nc.tensor.matmul(psums[1][:], lhsT, w[:, kb, 512:1024],
                                 start=st, stop=sp)

        def evict(act_fn, psums, out_tile, bias_tile, ev, tagp):
            for half in range(2):
                src = psums[half]
                sl = slice(half * 512, half * 512 + 512)
                if bias_tile is not None:
                    tmp = ev.tile([P, 512], dt.float32, tag=f"{tagp}{half}",
                                  bufs=1)
                    nc.vector.tensor_add(tmp[:], src[:], bias_tile[:, sl])
                    src = tmp
                nc.scalar.activation(out_tile[:, sl], src[:], act_fn)

        ld_bufs = 2 if (with_bias or "bf2" in tweak or "cat" in tweak) else 3
        sp_eng = nc.gpsimd if "sg" in tweak else nc.sync
        ps_bufs = 4 if "ps4" in tweak else 3
        bf_bufs = 2 if "bf2" in tweak else 1
        rb_bufs = 4 if "deep" in tweak else 2
        lt_bufs = 3 if "deep" in tweak else 2

        if layout == "p2":
            for _rep in range(repeat):
                # ---------------------------------------- phase A: z + r
                with ExitStack() as ph:
                  if phases == "all" or "a" in phases:
                    wz = load_weights_bf16([(wd["Wz"], 8), (wd["Uz"], 8), (wd["Cz"], 16)])
                    wr = load_weights_bf16([(wd["Wr"], 8), (wd["Ur"], 8), (wd["Cr"], 16)])
                    bz = load_bias("z")
                    br = load_bias("r")
                    ld = ph.enter_context(tc.tile_pool(name="lda", bufs=ld_bufs))
                    bf = ph.enter_context(tc.tile_pool(name="bfa", bufs=bf_bufs))
                    tp = ph.enter_context(tc.tile_pool(name="tpa", bufs=2))
                    ev = ph.enter_context(tc.tile_pool(name="eva", bufs=2))
                    ps = ph.enter_context(tc.tile_pool(name="psa", bufs=2, space="PSUM"))

                    for bt in range(n_bt):
                        r0 = bt * P
                        xh = ld.tile([P, 2048], dt.float32, tag="ld")
                        nc.sync.dma_start(xh[:, 0:1024], x_d[r0:r0 + P, :])
                        nc.sync.dma_start(xh[:, 1024:2048], h_d[r0:r0 + P, :])
                        aa = ld.tile([P, 2048], dt.float32, tag="ld")
                        nc.sync.dma_start(aa[:], a_d[r0:r0 + P, :])
                        xha_b = bf.tile([P, 4096], dt.bfloat16, tag="xha_b")
                        nc.vector.tensor_copy(xha_b[:, 0:2048], xh[:])
                        nc.scalar.activation(xha_b[:, 2048:4096], aa[:], AF.Copy)
                        xhaT = tp.tile([P, 32, P], dt.bfloat16, tag="xhaT")
                        nc.scalar.dma_start(xhaT[:], xha_b[:], transpose=True)

                        pz0 = ps.tile([P, 512], dt.float32, tag="pz0")
                        pz1 = ps.tile([P, 512], dt.float32, tag="pz1")
                        pr0 = ps.tile([P, 512], dt.float32, tag="pr0")
                        pr1 = ps.tile([P, 512], dt.float32, tag="pr1")
                        kbs = list(range(32))[::mm_stride]
                        first, last = kbs[0], kbs[-1]
                        for kb in kbs:
                            st = kb == first
                            sp = kb == last
                            lhsT = xhaT[:, kb, :]
                            nc.tensor.matmul(pz0[:], lhsT, wz[:, kb, 0:512],
                                             start=st, stop=sp)
                            nc.tensor.matmul(pz1[:], lhsT, wz[:, kb, 512:1024],
                                             start=st, stop=sp)
                            nc.tensor.matmul(pr0[:], lhsT, wr[:, kb, 0:512],
                                             start=st, stop=sp)
                            nc.tensor.matmul(pr1[:], lhsT, wr[:, kb, 512:1024],
                                             start=st, stop=sp)

                        z16 = ev.tile([P, HID], dt.float16, tag="z16")
                        evict(AF.Sigmoid, (pz0, pz1), z16, bz, ev, "zb")
                        rhb = ev.tile([P, HID], dt.bfloat16, tag="rhb")
                        for half, pp in enumerate((pr0, pr1)):
                            sl = slice(half * 512, half * 512 + 512)
                            if br is not None:
                                nc.vector.tensor_add(pp[:], pp[:], br[:, sl])
                            nc.scalar.activation(pp[:], pp[:], AF.Sigmoid)
                            nc.vector.tensor_mul(rhb[:, sl], pp[:], xh[:, 1024 + half * 512:1536 + half * 512])
                        rhT = tp.tile([P, 8, P], dt.bfloat16, tag="rhT")
                        nc.scalar.dma_start(rhT[:], rhb[:], transpose=True)
                        nc.sync.dma_start(xhaT_d[bt], xhaT[:])
                        nc.sync.dma_start(rhT_d[bt], rhT[:])
                        nc.sync.dma_start(z_d[bt], z16[:])

                # ---------------------------------------- phase B: s + t
                with ExitStack() as ph:
                  if phases == "all" or "b" in phases:
                    ws = load_weights_bf16(
                        [(wd["W"], 8), (wd["U"], 8), (wd["C"], 16)],
                        weng=nc.scalar if "wsc" in tweak else None)
                    wt = load_weights_bf16([(wd["Vo"], 8), (wd["Co"], 16), (wd["Uo"], 8)])
                    bs = load_bias("s")
                    btl = load_bias("t")
                    ld = ph.enter_context(tc.tile_pool(name="ldb", bufs=2))
                    tp = ph.enter_context(tc.tile_pool(name="tpb", bufs=2))
                    ev = ph.enter_context(tc.tile_pool(name="evb", bufs=2))
                    ps = ph.enter_context(tc.tile_pool(name="psb", bufs=2, space="PSUM"))

                    for bt in range(n_bt):
                        r0 = bt * P
                        lT = tp.tile([P, 32, P], dt.bfloat16, tag="lT",
                                     bufs=lt_bufs)
                        nc.sync.dma_start(lT[:, 0:8, :], xhaT_d[bt][:, 0:8, :])
                        nc.sync.dma_start(lT[:, 8:16, :], rhT_d[bt])
                        nc.sync.dma_start(lT[:, 16:32, :], xhaT_d[bt][:, 16:32, :])
                        z16 = ld.tile([P, HID], dt.float16, tag="z16")
                        nc.sync.dma_start(z16[:], z_d[bt])
                        hf = ld.tile([P, HID], dt.float32, tag="hf")
                        nc.sync.dma_start(hf[:], h_d[r0:r0 + P, :])

                        p0 = ps.tile([P, 512], dt.float32, tag="p0")
                        p1 = ps.tile([P, 512], dt.float32, tag="p1")
                        q0 = ps.tile([P, 512], dt.float32, tag="q0")
                        q1 = ps.tile([P, 512], dt.float32, tag="q1")
                        mm_kloop((p0, p1), lambda kb: lT[:, kb, :], ws, range(32))
                        mm_kloop((q0, q1),
                                 lambda kb: lT[:, kb if kb < 8 else kb + 8, :],
                                 wt, range(24), close=False)

                        stil = ev.tile([P, HID], dt.float32, tag="stil")
                        evict(AF.Tanh, (p0, p1), stil, bs, ev, "sb")
                        nc.vector.tensor_sub(stil[:], stil[:], hf[:])
                        nc.vector.tensor_mul(stil[:], z16[:], stil[:])
                        nc.vector.tensor_add(stil[:], hf[:], stil[:])
                        nc.sync.dma_start(s_out[r0:r0 + P, :], stil[:])

                        sb2 = ev.tile([P, HID], dt.bfloat16, tag="sb16")
                        nc.vector.tensor_copy(sb2[:], stil[:])
                        sT = tp.tile([P, 8, P], dt.bfloat16, tag="sT")
                        nc.scalar.dma_start(sT[:], sb2[:], transpose=True)
                        mm_kloop((q0, q1), lambda kb: sT[:, kb - 24, :], wt,
                                 range(24, 32), open_=False)
                        tf = ev.tile([P, HID], dt.float32, tag="tf")
                        evict(AF.Relu, (q0, q1), tf, btl, ev, "tb")
                        nc.sync.dma_start(t_out[r0:r0 + P, :], tf[:])


        if layout in ("p3", "p4"):
            for _rep3 in range(repeat):
                # ------------------------------------------------------- phase z
                with ExitStack() as ph:
                  if phases in ("all",) or "z" in phases:
                    w = load_weights_bf16([(wd["Wz"], 8), (wd["Uz"], 8), (wd["Cz"], 16)])
                    bz = load_bias("z")
                    ld = ph.enter_context(tc.tile_pool(name="ldz", bufs=ld_bufs))
                    bf = ph.enter_context(tc.tile_pool(name="bfz", bufs=bf_bufs))
                    tp = ph.enter_context(tc.tile_pool(name="tpz", bufs=2))
                    ev = ph.enter_context(tc.tile_pool(name="evz", bufs=2))
                    ps = ph.enter_context(tc.tile_pool(name="psz", bufs=ps_bufs, space="PSUM"))

                    for bt in range(n_bt):
                        r0 = bt * P
                        xh = ld.tile([P, 2048], dt.float32, tag="ld")
                        nc.sync.dma_start(xh[:, 0:1024], x_d[r0:r0 + P, :])
                        nc.sync.dma_start(xh[:, 1024:2048], h_d[r0:r0 + P, :])
                        aa = ld.tile([P, 2048], dt.float32, tag="ld")
                        nc.sync.dma_start(aa[:], a_d[r0:r0 + P, :])
                        xha_b = bf.tile([P, 4096], dt.bfloat16, tag="xha_b")
                        nc.vector.tensor_copy(xha_b[:, 0:2048], xh[:])
                        nc.scalar.activation(xha_b[:, 2048:4096], aa[:], AF.Copy)
                        xhaT = tp.tile([P, 32, P], dt.bfloat16, tag="xhaT")
                        nc.scalar.dma_start(xhaT[:], xha_b[:], transpose=True)

                        p0 = ps.tile([P, 512], dt.float32, tag="p0")
                        p1 = ps.tile([P, 512], dt.float32, tag="p1")
                        mm_kloop((p0, p1), lambda kb: xhaT[:, kb, :], w, range(32))

                        z16 = ev.tile([P, HID], dt.float16, tag="z16")
                        evict(AF.Sigmoid, (p0, p1), z16, bz, ev, "zb")
                        nc.sync.dma_start(xhaT_d[bt], xhaT[:])
                        nc.sync.dma_start(z_d[bt], z16[:])

                # ------------------------------------------------------- phase r
                with ExitStack() as ph:
                  if phases in ("all",) or "r" in phases:
                    w = load_weights_bf16([(wd["Wr"], 8), (wd["Ur"], 8), (wd["Cr"], 16)])
                    br = load_bias("r")
                    ld = ph.enter_context(tc.tile_pool(name="ldr", bufs=2))
                    tp = ph.enter_context(tc.tile_pool(name="tpr", bufs=2))
                    ev = ph.enter_context(tc.tile_pool(name="evr", bufs=2))
                    ps = ph.enter_context(tc.tile_pool(name="psr", bufs=ps_bufs, space="PSUM"))

                    for bt in range(n_bt):
                        r0 = bt * P
                        xhaT = tp.tile([P, 32, P], dt.bfloat16, tag="xhaT",
                                       bufs=rb_bufs)
                        nc.sync.dma_start(xhaT[:], xhaT_d[bt])
                        hf = ld.tile([P, HID], dt.float32, tag="hf",
                                     bufs=rb_bufs)
                        nc.sync.dma_start(hf[:], h_d[r0:r0 + P, :])

                        p0 = ps.tile([P, 512], dt.float32, tag="p0")
                        p1 = ps.tile([P, 512], dt.float32, tag="p1")
                        mm_kloop((p0, p1), lambda kb: xhaT[:, kb, :], w, range(32))

                        # sigmoid in place in PSUM, then rh = r * h straight out of
                        # PSUM into a bf16 tile.
                        rhb = ev.tile([P, HID], dt.bfloat16, tag="rhb")
                        for half, pp in enumerate((p0, p1)):
                            sl = slice(half * 512, half * 512 + 512)
                            if br is not None:
                                nc.vector.tensor_add(pp[:], pp[:], br[:, sl])
                            nc.scalar.activation(pp[:], pp[:], AF.Sigmoid)
                            nc.vector.tensor_mul(rhb[:, sl], pp[:], hf[:, sl])
                        rhT = tp.tile([P, 8, P], dt.bfloat16, tag="rhT")
                        nc.scalar.dma_start(rhT[:], rhb[:], transpose=True)
                        nc.sync.dma_start(rhT_d[bt], rhT[:])

                # ----------------------------------------------------- phase s+t
                with ExitStack() as ph:
                  if (phases in ("all",) or "s" in phases) and layout == "p3":
                    ws = load_weights_bf16([(wd["W"], 8), (wd["U"], 8), (wd["C"], 16)])
                    wt = load_weights_bf16([(wd["Vo"], 8), (wd["Co"], 16), (wd["Uo"], 8)])
                    bs = load_bias("s")
                    btl = load_bias("t")
                    ld = ph.enter_context(tc.tile_pool(name="lds", bufs=2))
                    tp = ph.enter_context(tc.tile_pool(name="tps", bufs=2))
                    ev = ph.enter_context(tc.tile_pool(name="evs", bufs=2))
                    ps = ph.enter_context(tc.tile_pool(name="pss", bufs=2, space="PSUM"))

                    for bt in range(n_bt):
                        r0 = bt * P
                        lT = tp.tile([P, 32, P], dt.bfloat16, tag="lT")
                        nc.sync.dma_start(lT[:, 0:8, :], xhaT_d[bt][:, 0:8, :])
                        nc.sync.dma_start(lT[:, 8:16, :], rhT_d[bt])
                        nc.sync.dma_start(lT[:, 16:32, :], xhaT_d[bt][:, 16:32, :])
                        z16 = ld.tile([P, HID], dt.float16, tag="z16")
                        nc.sync.dma_start(z16[:], z_d[bt])
                        hf = ld.tile([P, HID], dt.float32, tag="hf")
                        nc.sync.dma_start(hf[:], h_d[r0:r0 + P, :])

                        p0 = ps.tile([P, 512], dt.float32, tag="p0")
                        p1 = ps.tile([P, 512], dt.float32, tag="p1")
                        q0 = ps.tile([P, 512], dt.float32, tag="q0")
                        q1 = ps.tile([P, 512], dt.float32, tag="q1")
                        mm_kloop((p0, p1), lambda kb: lT[:, kb, :], ws, range(32))
                        mm_kloop((q0, q1),
                                 lambda kb: lT[:, kb if kb < 8 else kb + 8, :],
                                 wt, range(24), close=False)

                        stil = ev.tile([P, HID], dt.float32, tag="stil")
                        evict(AF.Tanh, (p0, p1), stil, bs, ev, "sb")
                        nc.vector.tensor_sub(stil[:], stil[:], hf[:])
                        nc.vector.tensor_mul(stil[:], z16[:], stil[:])
                        nc.vector.tensor_add(stil[:], hf[:], stil[:])
                        nc.sync.dma_start(s_out[r0:r0 + P, :], stil[:])

                        sb2 = ev.tile([P, HID], dt.bfloat16, tag="sb16")
                        nc.vector.tensor_copy(sb2[:], stil[:])
                        sT = tp.tile([P, 8, P], dt.bfloat16, tag="sT")
                        nc.scalar.dma_start(sT[:], sb2[:], transpose=True)
                        mm_kloop((q0, q1), lambda kb: sT[:, kb - 24, :], wt,
                                 range(24, 32), open_=False)
                        tf = ev.tile([P, HID], dt.float32, tag="tf")
                        evict(AF.Relu, (q0, q1), tf, btl, ev, "tb")
                        nc.sync.dma_start(t_out[r0:r0 + P, :], tf[:])

                # ----------------------------------------------------- p4: phase s
                with ExitStack() as ph:
                  if (phases in ("all",) or "s" in phases) and layout == "p4":
                    ws = load_weights_bf16([(wd["W"], 8), (wd["U"], 8), (wd["C"], 16)])
                    bs = load_bias("s")
                    ld = ph.enter_context(tc.tile_pool(name="lds4", bufs=2))
                    tp = ph.enter_context(tc.tile_pool(name="tps4", bufs=2))
                    ev = ph.enter_context(tc.tile_pool(name="evs4", bufs=2))
                    ps = ph.enter_context(tc.tile_pool(name="pss4", bufs=3, space="PSUM"))

                    for bt in range(n_bt):
                        r0 = bt * P
                        lT = tp.tile([P, 32, P], dt.bfloat16, tag="lT")
                        nc.sync.dma_start(lT[:, 0:8, :], xhaT_d[bt][:, 0:8, :])
                        nc.sync.dma_start(lT[:, 8:16, :], rhT_d[bt])
                        nc.sync.dma_start(lT[:, 16:32, :], xhaT_d[bt][:, 16:32, :])
                        z16 = ld.tile([P, HID], dt.float16, tag="z16")
                        nc.sync.dma_start(z16[:], z_d[bt])
                        hf = ld.tile([P, HID], dt.float32, tag="hf")
                        nc.sync.dma_start(hf[:], h_d[r0:r0 + P, :])

                        p0 = ps.tile([P, 512], dt.float32, tag="p0")
                        p1 = ps.tile([P, 512], dt.float32, tag="p1")
                        mm_kloop((p0, p1), lambda kb: lT[:, kb, :], ws, range(32))

                        stil = ev.tile([P, HID], dt.float32, tag="stil")
                        evict(AF.Tanh, (p0, p1), stil, bs, ev, "sb")
                        nc.vector.tensor_sub(stil[:], stil[:], hf[:])
                        nc.vector.tensor_mul(stil[:], z16[:], stil[:])
                        nc.vector.tensor_add(stil[:], hf[:], stil[:])
                        nc.sync.dma_start(s_out[r0:r0 + P, :], stil[:])
                        sb2 = ev.tile([P, HID], dt.bfloat16, tag="sb16")
                        nc.vector.tensor_copy(sb2[:], stil[:])
                        sT = tp.tile([P, 8, P], dt.bfloat16, tag="sT")
                        nc.scalar.dma_start(sT[:], sb2[:], transpose=True)
                        nc.sync.dma_start(sT_d[bt], sT[:])

                # ----------------------------------------------------- p4: phase t
                with ExitStack() as ph:
                  if (phases in ("all",) or "t" in phases) and layout == "p4":
                    wt = load_weights_bf16([(wd["Vo"], 8), (wd["Co"], 16), (wd["Uo"], 8)])
                    btl = load_bias("t")
                    tp = ph.enter_context(tc.tile_pool(name="tpt4", bufs=2))
                    ev = ph.enter_context(tc.tile_pool(name="evt4", bufs=2))
                    ps = ph.enter_context(tc.tile_pool(name="pst4", bufs=3, space="PSUM"))

                    for bt in range(n_bt):
                        r0 = bt * P
                        lT = tp.tile([P, 32, P], dt.bfloat16, tag="lT2")
                        nc.sync.dma_start(lT[:, 0:8, :], xhaT_d[bt][:, 0:8, :])
                        nc.sync.dma_start(lT[:, 8:24, :], xhaT_d[bt][:, 16:32, :])
                        nc.sync.dma_start(lT[:, 24:32, :], sT_d[bt])

                        q0 = ps.tile([P, 512], dt.float32, tag="q0")
                        q1 = ps.tile([P, 512], dt.float32, tag="q1")
                        mm_kloop((q0, q1), lambda kb: lT[:, kb, :], wt, range(32))

                        tf = ev.tile([P, HID], dt.float32, tag="tf")
                        evict(AF.Relu, (q0, q1), tf, btl, ev, "tb")
                        nc.sync.dma_start(t_out[r0:r0 + P, :], tf[:])

    nc.compile()
    return nc


_CACHE = {}


def _get_exec(n_bt, with_bias, mm_stride=1, wload="hostbf16", layout="p3", repeat=1, phases="all", tweak=""):
    """Build (once per process) the compiled program and a sharded jit
    callable over the 8 cores, mirroring bass2jax.run_bass_via_pjrt."""
    key = (n_bt, with_bias, mm_stride, wload, layout, repeat, phases, tweak)
    if key in _CACHE:
        return _CACHE[key]

    import jax
    import concourse.mybir as mybir
    from concourse import bass2jax
    from jax.sharding import Mesh, PartitionSpec
    from jax.experimental.shard_map import shard_map

    bass2jax.install_neuronx_cc_hook()
    nc = _build_nc(n_bt, with_bias, mm_stride, wload, layout, repeat, phases, tweak)

    partition_name = (nc.partition_id_tensor.name
                      if nc.partition_id_tensor else None)
    in_names = []
    out_names = []
    out_avals = []
    zero_outs = []
    for alloc in nc.m.functions[0].allocations:
        if not isinstance(alloc, mybir.MemoryLocationSet):
            continue
        name = alloc.memorylocations[0].name
        if alloc.kind == "ExternalInput":
            if name != partition_name:
                in_names.append(name)
        elif alloc.kind == "ExternalOutput":
            out_names.append(name)
            shape = tuple(alloc.tensor_shape)
            dtype = mybir.dt.np(alloc.dtype)
            out_avals.append(jax.core.ShapedArray(shape, dtype))
            zero_outs.append(np.zeros(shape, dtype))
    n_params = len(in_names)
    all_in_names = in_names + out_names
    if partition_name is not None:
        all_in_names = all_in_names + [partition_name]

    def _body(*args):
        operands = list(args)
        if partition_name is not None:
            operands.append(bass2jax.partition_id_tensor())
        outs = bass2jax._bass_exec_p.bind(
            *operands,
            out_avals=tuple(out_avals),
            in_names=tuple(all_in_names),
            out_names=tuple(out_names),
            lowering_input_output_aliases=(),
            sim_require_finite=True,
            sim_require_nnan=True,
            nc=nc,
        )
        return tuple(outs)

    devices = jax.devices()[:N_CORES]
    mesh = Mesh(np.asarray(devices), ("core",))
    n_outs = len(out_names)
    sharded = jax.jit(
        shard_map(
            _body, mesh=mesh,
            in_specs=(PartitionSpec("core"),) * (n_params + n_outs),
            out_specs=(PartitionSpec("core"),) * n_outs,
            check_rep=False,
        ),
        keep_unused=True,
    )
    entry = {
        "wload": wload,
        "nc": nc,
        "sharded": sharded,
        "in_names": in_names,
        "out_names": out_names,
        "zero_outs": zero_outs,
        "mesh": mesh,
    }
    _CACHE[key] = entry
    return entry


def _prepare_in_arrays(entry, inputs, bias_rows):
    """Concatenated (8*shape[0], ...) global arrays in BIR input order."""
    per_core = {
        "x": inputs["in_word"],
        "h": inputs["last_hid_state"],
        "a": inputs["attended_state"],
    }
    if any(n == "xha" for n in entry["in_names"]):
        per_core["xha"] = np.concatenate(
            [np.asarray(inputs["in_word"], np.float32),
             np.asarray(inputs["last_hid_state"], np.float32),
             np.asarray(inputs["attended_state"], np.float32)], axis=1)
    arrs = []
    for name in entry["in_names"]:
        if name in per_core:
            arrs.append(np.ascontiguousarray(per_core[name], dtype=np.float32))
        elif name.startswith("bias_"):
            g = name.split("_")[1]
            row = np.broadcast_to(np.asarray(bias_rows[g], np.float32), (P, HID))
            arrs.append(np.ascontiguousarray(np.tile(row, (N_CORES, 1))))
        else:
            w = np.asarray(inputs[name], dtype=np.float32)
            if entry.get("wload") == "hostbf16":
                import ml_dtypes
                w = w.astype(ml_dtypes.bfloat16)
            arrs.append(np.ascontiguousarray(np.tile(w, (N_CORES, 1))))
    return arrs


def kernel(in_word, last_hid_state, attended_state,
           W, bw, Wz, bwz, Wr, bwr,
           U, bu, Uz, buz, Ur, bur,
           C, bc, Cz, bcz, Cr, bcr,
           Uo, buo, Vo, bvo, Co, bco):
    inputs = dict(in_word=np.asarray(in_word),
                  last_hid_state=np.asarray(last_hid_state),
                  attended_state=np.asarray(attended_state),
                  W=W, Wz=Wz, Wr=Wr, U=U, Uz=Uz, Ur=Ur,
                  C=C, Cz=Cz, Cr=Cr, Uo=Uo, Vo=Vo, Co=Co)
    bias_rows = {
        "z": np.asarray(bwz) + np.asarray(buz) + np.asarray(bcz),
        "r": np.asarray(bwr) + np.asarray(bur) + np.asarray(bcr),
        "s": np.asarray(bw) + np.asarray(bu) + np.asarray(bc),
        "t": np.asarray(buo) + np.asarray(bvo) + np.asarray(bco),
    }
    with_bias = any(np.any(v != 0) for v in bias_rows.values())

    entry = _get_exec(N_BT_FULL, with_bias)
    arrs = _prepare_in_arrays(entry, inputs, bias_rows)
    zeros = [np.zeros((N_CORES * z.shape[0], *z.shape[1:]), z.dtype)
             for z in entry["zero_outs"]]
    outs = entry["sharded"](*arrs, *zeros)
    res = {name: np.asarray(outs[i]) for i, name in enumerate(entry["out_names"])}
    return (res["s"], res["t"])

